# revision 1
# baseline (speedup 1.0000x reference)
"""CorrelationHead Trainium2 kernel.

Math: SpatialCorrelationSampler(patch=16, dil=2) on 7x7 maps zero-pads x2 by
(14,16). The displaced sample x2pad[i+2ph, j+2pw] is nonzero only when it
lands in the true 7x7 patch, so corr[b] (12544 features) has exactly 2401
distinct nonzero values = Gram matrix G[b][ij,kl] = sum_c x1[b,c,ij]*x2[b,c,kl]
(valid only when parity matches; invalid ones never appear in corr).
fc1(corr) therefore equals sum_{ij,kl} G[b][ij,kl] * W1eff[kl,ij,rep] with
W1eff[kl,ij,:] = W1[:, ((ph*16+pw)*49+ij)] for parity-valid (ij,kl), else 0.

Sharding: pure data-parallel over the 1024 RoIs -> 128 per each of 8 cores.
Weights replicated. Each core: per-RoI Gram matmuls on PE (K=256 contraction,
2 accumulating matmuls), evict to SBUF, then fc1 as 49 accumulated K=49
matmuls, bias via ones-row matmul, ReLU on ACT, PE-transpose, fc2, ReLU,
transpose, fc3.
"""

import os
import numpy as np

import concourse.bass as bass
import concourse.mybir as mybir
from concourse.bass_utils import run_bass_kernel_spmd

# ---------------------------------------------------------------- constants
P = 16
DIL = 2
H = 7
C = 256
B = 1024
REP = 1024
HW = H * H  # 49
N_CORES = 8
BL = B // N_CORES  # 128 RoIs per core

CH = 16                 # RoIs per input chunk
NCHUNK = BL // CH       # 8
NG = 4                  # RoIs per PSUM gram group
NGROUP = BL // NG       # 32
GPC = CH // NG          # groups per chunk = 4
PSG_RING = 3

SL = 8                  # fc1 ij's per streamed W1 slice
SLICES = [8, 8, 8, 8, 8, 8, 1]   # 49 ijs
NSLICE = len(SLICES)

F32 = mybir.dt.float32
BF16 = mybir.dt.bfloat16

# dtype of matmul operands on device ("f32" or "bf16")
DT_MODE = os.environ.get("CORR_DT", "bf16")
EVICT_MODE = os.environ.get("CORR_EVICT", "perb")  # "perb" | "batch" | "split"
W1_ENG = os.environ.get("CORR_W1ENG", "act")       # "act" | "gpsimd" | "sp"

LAST_EXEC_NS = None
_CACHE = {}


# ---------------------------------------------------------------- host prep
def _w1eff(W1, np_dt):
    """[49 kl, 49 ij, 1024] with zeros for parity-invalid (ij,kl)."""
    w = np.zeros((HW, HW, REP), dtype=np_dt)
    for i in range(H):
        for j in range(H):
            ij = i * H + j
            for k in range(H):
                if (k - i) % 2:
                    continue
                ph = (k - i) // 2 + 7
                for l in range(H):
                    if (l - j) % 2:
                        continue
                    pw = (l - j) // 2 + 7
                    kl = k * H + l
                    f = (ph * P + pw) * HW + ij
                    w[kl, ij, :] = W1[:, f]
    return w


# ---------------------------------------------------------------- device IR
def _build(dt, debug=False):
    nc = bass.Bass()

    x1h = nc.dram_tensor("x1h", [2, 128, BL * HW], dt, kind="ExternalInput")
    x2h = nc.dram_tensor("x2h", [2, 128, BL * HW], dt, kind="ExternalInput")
    w1h = nc.dram_tensor("w1h", [HW, HW * REP], dt, kind="ExternalInput")
    w2h = nc.dram_tensor("w2h", [128, 8 * REP], dt, kind="ExternalInput")
    w3h = nc.dram_tensor("w3h", [128, 8 * 4], dt, kind="ExternalInput")
    b1h = nc.dram_tensor("b1h", [1, REP], dt, kind="ExternalInput")
    b2h = nc.dram_tensor("b2h", [1, REP], dt, kind="ExternalInput")
    b3h = nc.dram_tensor("b3h", [1, 4], dt, kind="ExternalInput")
    onesh = nc.dram_tensor("onesh", [1, 128], dt, kind="ExternalInput")
    identh = nc.dram_tensor("identh", [128, 128], dt, kind="ExternalInput")
    zbh = nc.dram_tensor("zbh", [128, 1], F32, kind="ExternalInput")
    outh = nc.dram_tensor("outh", [128, 4], F32, kind="ExternalOutput")
    if debug:
        gsadbg = nc.dram_tensor("gsadbg", [HW, HW, 128], dt, kind="ExternalOutput")
        relu1dbg = nc.dram_tensor("relu1dbg", [128, REP], dt, kind="ExternalOutput")
        r1Tdbg = nc.dram_tensor("r1Tdbg", [128, REP], dt, kind="ExternalOutput")
        relu2dbg = nc.dram_tensor("relu2dbg", [128, REP], dt, kind="ExternalOutput")

    CW = CH * HW  # 784 columns per chunk

    from contextlib import ExitStack

    with ExitStack() as ctx:
        sb = lambda name, shape, d: ctx.enter_context(nc.sbuf_tensor(name, shape, d))
        ps = lambda name, shape, d: ctx.enter_context(nc.psum_tensor(name, shape, d))
        sem = lambda name: ctx.enter_context(nc.semaphore(name))

        x1s0 = sb("x1s0", [128, 2, CW], dt)
        x1s1 = sb("x1s1", [128, 2, CW], dt)
        x2s0 = sb("x2s0", [128, 2, CW], dt)
        x2s1 = sb("x2s1", [128, 2, CW], dt)
        gsa = sb("gsa", [HW, HW, 128], dt)
        w1_resident = dt != F32
        if w1_resident:
            w1r = sb("w1r", [HW, HW * REP], dt)
        else:
            w1s0 = sb("w1s0", [HW, SL * REP], dt)
            w1s1 = sb("w1s1", [HW, SL * REP], dt)
        w2s = sb("w2s", [128, 8 * REP], dt)
        w3s = sb("w3s", [128, 8 * 4], dt)
        b1s = sb("b1s", [1, REP], dt)
        b2s = sb("b2s", [1, REP], dt)
        b3s = sb("b3s", [1, 4], dt)
        ones = sb("ones", [1, 128], dt)
        idents = sb("idents", [128, 128], dt)
        zbias = sb("zbias", [128, 1], F32)
        relu1 = sb("relu1", [128, REP], dt)
        r1T = sb("r1T", [128, REP], dt)
        relu2 = sb("relu2", [128, REP], dt)
        r2T = sb("r2T", [128, REP], dt)
        outs = sb("outs", [128, 4], F32)
        psG0 = ps("psG0", [HW, NG, HW], F32)
        psG1 = ps("psG1", [HW, NG, HW], F32)
        psG2 = ps("psG2", [HW, NG, HW], F32)
        psF0 = ps("psF0", [128, 512], F32)
        psF1 = ps("psF1", [128, 512], F32)
        psT0 = ps("psT0", [128, 128], dt)
        psT1 = ps("psT1", [128, 128], dt)
        psO = ps("psO", [128, 4], F32)
        s_x0 = sem("s_x0")
        s_x1 = sem("s_x1")
        s_xd = sem("s_xd")
        s_w = sem("s_w")
        s_w1a = sem("s_w1a")
        s_w1b = sem("s_w1b")
        s_w1u = sem("s_w1u")
        s_g = sem("s_g")
        s_e = sem("s_e")
        s_ed = sem("s_ed")
        s_f1 = sem("s_f1")
        s_r1 = sem("s_r1")
        s_t1 = sem("s_t1")
        s_c1 = sem("s_c1")
        s_f2 = sem("s_f2")
        s_r2 = sem("s_r2")
        s_t2 = sem("s_t2")
        s_c2 = sem("s_c2")
        s_f3 = sem("s_f3")
        s_oe = sem("s_oe")
        s_o = sem("s_o")
        block = ctx.enter_context(nc.Block())
        x1s = [x1s0, x1s1]
        x2s = [x2s0, x2s1]
        w1s = None if w1_resident else [w1s0, w1s1]
        sxs = [s_x0, s_x1]
        sw1s = [s_w1a, s_w1b]
        psG = [psG0, psG1, psG2]
        psF = [psF0, psF1]
        psT = [psT0, psT1]
        N_WDMA = 8 * 16  # 8 initial weight DMAs

        # ---------------- SP: input + weight DMAs, final output
        @block.sync
        def _(sp):
            sp.dma_start(w2s[:, :], w2h[:, :]).then_inc(s_w, 16)
            sp.dma_start(w3s[:, :], w3h[:, :]).then_inc(s_w, 16)
            sp.dma_start(b1s[:, :], b1h[:, :]).then_inc(s_w, 16)
            sp.dma_start(b2s[:, :], b2h[:, :]).then_inc(s_w, 16)
            sp.dma_start(b3s[:, :], b3h[:, :]).then_inc(s_w, 16)
            sp.dma_start(ones[:, :], onesh[:, :]).then_inc(s_w, 16)
            sp.dma_start(idents[:, :], identh[:, :]).then_inc(s_w, 16)
            sp.dma_start(zbias[:, :], zbh[:, :]).then_inc(s_w, 16)
            if w1_resident and W1_ENG == "sp":
                _w1_dmas(sp)
            for ch in range(NCHUNK):
                sl = ch % 2
                if ch >= 2:
                    sp.wait_ge(s_xd, ch - 1)
                lo, hi = ch * CW, (ch + 1) * CW
                sp.dma_start(x1s[sl][:, 0, :], x1h[0, :, lo:hi]).then_inc(sxs[sl], 16)
                sp.dma_start(x1s[sl][:, 1, :], x1h[1, :, lo:hi]).then_inc(sxs[sl], 16)
                sp.dma_start(x2s[sl][:, 0, :], x2h[0, :, lo:hi]).then_inc(sxs[sl], 16)
                sp.dma_start(x2s[sl][:, 1, :], x2h[1, :, lo:hi]).then_inc(sxs[sl], 16)
            if debug:
                sp.wait_ge(s_e, NGROUP)
                sp.dma_start(gsadbg[:, :, :], gsa[:, :, :]).then_inc(s_o, 16)
                sp.wait_ge(s_r1, 2)
                sp.dma_start(relu1dbg[:, :], relu1[:, :]).then_inc(s_o, 16)
                sp.wait_ge(s_c1, 8)
                sp.dma_start(r1Tdbg[:, :], r1T[:, :]).then_inc(s_o, 16)
                sp.wait_ge(s_r2, 2)
                sp.dma_start(relu2dbg[:, :], relu2[:, :]).then_inc(s_o, 16)
            sp.wait_ge(s_oe, 1)
            sp.dma_start(outh[:, :], outs[:, :]).then_inc(s_o, 16)
            sp.wait_ge(s_o, 16 + (64 if debug else 0))

        def _w1_dmas(eng):
            # 4 big DMAs; transfers overlap the Gram phase
            for s in range(4):
                lo = s * 13 * REP
                hi = min(HW * REP, (s + 1) * 13 * REP)
                eng.dma_start(w1r[:, lo:hi], w1h[:, lo:hi]).then_inc(s_w1a, 16)

        # ---------------- GPSIMD: W1eff load / f32 streaming
        if not w1_resident:

            @block.gpsimd
            def _(gp):
                for s in range(NSLICE):
                    slot = s % 2
                    if s >= 2:
                        gp.wait_ge(s_w1u, s - 1)
                    ncols = SLICES[s] * REP
                    gp.dma_start(
                        w1s[slot][:, 0:ncols],
                        w1h[:, s * SL * REP : s * SL * REP + ncols],
                    ).then_inc(sw1s[slot], 16)
        elif W1_ENG == "gpsimd":

            @block.gpsimd
            def _(gp):
                _w1_dmas(gp)

        # ---------------- PE: all matmuls
        @block.tensor
        def _(pe):
            # Gram phase: G[b]^T[kl, ij] for each local RoI
            for ch in range(NCHUNK):
                sl = ch % 2
                pe.wait_ge(sxs[sl], 64 * (ch // 2 + 1))
                for g in range(GPC):
                    gi = ch * GPC + g
                    q = gi % PSG_RING
                    if gi >= PSG_RING:
                        pe.wait_ge(s_e, gi - PSG_RING + 1)
                        if EVICT_MODE == "split":
                            pe.wait_ge(s_ed, gi - PSG_RING + 1)
                    for bb in range(NG):
                        lb = g * NG + bb
                        for t in range(2):
                            mm = pe.matmul(
                                psG[q][:, bb, :],
                                x2s[sl][:, t, lb * HW : (lb + 1) * HW],
                                x1s[sl][:, t, lb * HW : (lb + 1) * HW],
                                start=(t == 0),
                                stop=(t == 1),
                            )
                    mm.then_inc(s_g, 1)

            # fc1: out1[b,rep] = sum_ij sum_kl G^T[kl, ij*128+b] * W1eff
            pe.wait_ge(s_e, NGROUP)
            if EVICT_MODE == "split":
                pe.wait_ge(s_ed, NGROUP)
            pe.wait_ge(s_w, N_WDMA)
            if w1_resident:
                pe.wait_ge(s_w1a, 64)
                for ij in range(HW):
                    for hf in range(2):
                        pe.matmul(
                            psF[hf][:, :],
                            gsa[:, ij, :],
                            w1r[:, ij * REP + hf * 512 : ij * REP + hf * 512 + 512],
                            start=(ij == 0),
                            stop=False,
                        )
            else:
                for s in range(NSLICE):
                    slot = s % 2
                    pe.wait_ge(sw1s[slot], 16 * (s // 2 + 1))
                    for j in range(SLICES[s]):
                        ij = s * SL + j
                        for hf in range(2):
                            mm = pe.matmul(
                                psF[hf][:, :],
                                gsa[:, ij, :],
                                w1s[slot][
                                    :, j * REP + hf * 512 : j * REP + hf * 512 + 512
                                ],
                                start=(ij == 0),
                                stop=False,
                            )
                    mm.then_inc(s_w1u, 1)
            for hf in range(2):
                pe.matmul(
                    psF[hf][:, :],
                    ones[:, :],
                    b1s[:, hf * 512 : hf * 512 + 512],
                    start=False,
                    stop=True,
                ).then_inc(s_f1, 1)

            # transpose relu1 -> r1T
            pe.wait_ge(s_r1, 2)
            for k in range(8):
                if k >= 2:
                    pe.wait_ge(s_c1, k - 1)
                pe.transpose(
                    psT[k % 2][:, :], relu1[:, k * 128 : (k + 1) * 128], idents[:, :]
                ).then_inc(s_t1, 1)

            # fc2
            for k in range(8):
                pe.wait_ge(s_c1, k + 1)
                for hf in range(2):
                    pe.matmul(
                        psF[hf][:, :],
                        r1T[:, k * 128 : (k + 1) * 128],
                        w2s[:, k * REP + hf * 512 : k * REP + hf * 512 + 512],
                        start=(k == 0),
                        stop=False,
                    )
            for hf in range(2):
                pe.matmul(
                    psF[hf][:, :],
                    ones[:, :],
                    b2s[:, hf * 512 : hf * 512 + 512],
                    start=False,
                    stop=True,
                ).then_inc(s_f2, 1)

            # transpose relu2 -> r2T
            pe.wait_ge(s_r2, 2)
            for k in range(8):
                if k >= 2:
                    pe.wait_ge(s_c2, k - 1)
                pe.transpose(
                    psT[k % 2][:, :], relu2[:, k * 128 : (k + 1) * 128], idents[:, :]
                ).then_inc(s_t2, 1)

            # fc3
            for k in range(8):
                pe.wait_ge(s_c2, k + 1)
                pe.matmul(
                    psO[:, :],
                    r2T[:, k * 128 : (k + 1) * 128],
                    w3s[:, k * 4 : (k + 1) * 4],
                    start=(k == 0),
                    stop=False,
                )
            pe.matmul(
                psO[:, :], ones[:, :], b3s[:, :], start=False, stop=True
            ).then_inc(s_f3, 1)

        # ---------------- ACT: W1 load (own HWDGE ring), PSUM evictions + ReLU
        @block.scalar
        def _(act):
            if w1_resident and W1_ENG == "act":
                _w1_dmas(act)
            for gi in range(NGROUP):
                q = gi % PSG_RING
                act.wait_ge(s_g, gi + 1)
                if EVICT_MODE == "batch":
                    act.activation(
                        gsa[:, :, gi * NG : (gi + 1) * NG],
                        psG[q][:, :, :].rearrange("p b i -> p i b"),
                        mybir.ActivationFunctionType.Copy,
                    ).then_inc(s_e, 1)
                else:
                    bbs = range(2) if EVICT_MODE == "split" else range(NG)
                    for bb in bbs:
                        b = gi * NG + bb
                        last = act.activation(
                            gsa[:, :, b], psG[q][:, bb, :],
                            mybir.ActivationFunctionType.Copy,
                        )
                    last.then_inc(s_e, 1)
                if gi % GPC == GPC - 1:
                    # s_g >= gi+1 proves the chunk's PE matmuls completed, so
                    # its x slot may be overwritten by SP
                    act.nop().then_inc(s_xd, 1)
            act.wait_ge(s_w, N_WDMA)
            for hf in range(2):
                act.wait_ge(s_f1, hf + 1)
                act.activation(
                    relu1[:, hf * 512 : (hf + 1) * 512], psF[hf][:, :],
                    mybir.ActivationFunctionType.Relu, bias=zbias[:, :],
                ).then_inc(s_r1, 1)
            for hf in range(2):
                act.wait_ge(s_f2, hf + 1)
                act.activation(
                    relu2[:, hf * 512 : (hf + 1) * 512], psF[hf][:, :],
                    mybir.ActivationFunctionType.Relu, bias=zbias[:, :],
                ).then_inc(s_r2, 1)
            act.wait_ge(s_f3, 1)
            act.activation(
                outs[:, :], psO[:, :], mybir.ActivationFunctionType.Copy
            ).then_inc(s_oe, 1)

        # ---------------- DVE: split evictions + transpose copybacks
        @block.vector
        def _(dve):
            if EVICT_MODE == "split":
                for gi in range(NGROUP):
                    q = gi % PSG_RING
                    dve.wait_ge(s_g, gi + 1)
                    for bb in range(2, NG):
                        b = gi * NG + bb
                        last = dve.tensor_copy(gsa[:, :, b], psG[q][:, bb, :])
                    last.then_inc(s_ed, 1)
            for k in range(8):
                dve.wait_ge(s_t1, k + 1)
                dve.tensor_copy(
                    r1T[:, k * 128 : (k + 1) * 128], psT[k % 2][:, :]
                ).then_inc(s_c1, 1)
            for k in range(8):
                dve.wait_ge(s_t2, k + 1)
                dve.tensor_copy(
                    r2T[:, k * 128 : (k + 1) * 128], psT[k % 2][:, :]
                ).then_inc(s_c2, 1)

    return nc


def _get_nc(dt):
    key = ("nc", str(dt))
    if key not in _CACHE:
        _CACHE[key] = _build(dt)
    return _CACHE[key]


# ---------------------------------------------------------------- entry
def kernel(patch1, patch2, W1, b1, W2, b2, W3, b3):
    global LAST_EXEC_NS
    dt = BF16 if DT_MODE == "bf16" else F32
    np_dt = np.float32 if dt == F32 else None  # bf16 handled via ml_dtypes

    if dt == BF16:
        import ml_dtypes
        np_dt = ml_dtypes.bfloat16

    patch1 = np.asarray(patch1, dtype=np.float32).reshape(B, C, HW)
    patch2 = np.asarray(patch2, dtype=np.float32).reshape(B, C, HW)
    W1 = np.asarray(W1, dtype=np.float32)
    W2 = np.asarray(W2, dtype=np.float32)
    W3 = np.asarray(W3, dtype=np.float32)
    b1 = np.asarray(b1, dtype=np.float32)
    b2 = np.asarray(b2, dtype=np.float32)
    b3 = np.asarray(b3, dtype=np.float32)

    w1e = _w1eff(W1, np_dt).reshape(HW, HW * REP)
    w2e = np.ascontiguousarray(
        W2.T.reshape(8, 128, REP).transpose(1, 0, 2).reshape(128, 8 * REP)
    ).astype(np_dt)
    w3e = np.ascontiguousarray(
        W3.T.reshape(8, 128, 4).transpose(1, 0, 2).reshape(128, 32)
    ).astype(np_dt)

    shared = {
        "w1h": np.ascontiguousarray(w1e).astype(np_dt, copy=False),
        "w2h": w2e,
        "w3h": w3e,
        "b1h": b1.reshape(1, REP).astype(np_dt),
        "b2h": b2.reshape(1, REP).astype(np_dt),
        "b3h": b3.reshape(1, 4).astype(np_dt),
        "onesh": np.ones((1, 128), dtype=np_dt),
        "identh": np.eye(128, dtype=np.float32).astype(np_dt),
        "zbh": np.zeros((128, 1), dtype=np.float32),
    }

    in_maps = []
    for i in range(N_CORES):
        sl = slice(i * BL, (i + 1) * BL)
        x1 = np.ascontiguousarray(
            patch1[sl].transpose(1, 0, 2).reshape(2, 128, BL * HW)
        ).astype(np_dt)
        x2 = np.ascontiguousarray(
            patch2[sl].transpose(1, 0, 2).reshape(2, 128, BL * HW)
        ).astype(np_dt)
        in_maps.append({"x1h": x1, "x2h": x2, **shared})

    nc = _get_nc(dt)
    trace = os.environ.get("CORR_TRACE", "0") == "1"
    res = run_bass_kernel_spmd(nc, in_maps, list(range(N_CORES)), trace=trace)
    LAST_EXEC_NS = res.exec_time_ns

    out = np.concatenate(
        [res.results[i]["outh"] for i in range(N_CORES)], axis=0
    ).astype(np.float32)
    return out



# revision 9
# speedup vs baseline: 2.4113x; 2.4113x over previous
"""CorrelationHead Trainium2 kernel (v2: parity-stacked fc1).

Math: SpatialCorrelationSampler(patch=16, dil=2) on 7x7 maps zero-pads x2 by
(14,16). corr[b] (12544 features) has exactly 2401 distinct nonzero values =
Gram matrix G[b][kl,ij] = sum_c x1[b,c,ij]*x2[b,c,kl], and only parity-valid
(kl,ij) pairs (k=i mod 2, l=j mod 2) ever appear in corr: 625 of 2401.
fc1(corr) = sum over the 625 valid pairs of G * W1eff, so we:
  1. gram:  per-RoI G^T[kl, ij] on PE (K=256 contraction, 2 matmuls),
     evict to SBUF gsa[49, 128b, 49ij] (DVE).
  2. stack: 49 selection matmuls (0/1 Sel, M=32-row slot, N=128b) scatter the
     625 valid (kl,ij) rows into 6 dense K<=128 "stacks" in PSUM, 2-3 ij per
     32-slot via PSUM accumulation. b1 folds in as a constant ones-row.
  3. fc1:   12 matmuls (6 stacks x 2 psum halves) with densely packed W1
     (1.57MB instead of 49x49x1024 = 4.9MB with 74% structural zeros).
  4. tail:  PE transpose + fc2 + transpose + fc3 (as v1).

DMA: x (6.4MB bf16) is the floor; 16 chunks alternate the two HWDGE queues
(SP/ACT) with 3.1KB per-partition contiguous runs; w2 halves trail x on the
same queues; Sel/W1/consts ride the gpsimd SWDGE queue.

Sharding: pure data-parallel over the 1024 RoIs -> 128 per each of 8 cores.
"""

import os
import numpy as np

import concourse.bass as bass
import concourse.mybir as mybir
from concourse.bass_utils import run_bass_kernel_spmd

# ---------------------------------------------------------------- constants
P = 16
DIL = 2
H = 7
C = 256
B = 1024
REP = 1024
HW = H * H  # 49
N_CORES = 8
BL = B // N_CORES  # 128 RoIs per core

NCHUNK = 16
CB = BL // NCHUNK        # 8 RoIs per chunk
NG = 4                   # RoIs per PSUM gram group
NGROUP = BL // NG        # 32
GPC = CB // NG           # groups per chunk = 2
CHW = 2 * 2 * CB * HW    # els per partition per chunk = 1568

F32 = mybir.dt.float32
BF16 = mybir.dt.bfloat16

LAST_EXEC_NS = None
_CACHE = {}


# ------------------------------------------------------------- stack layout
def _klist(ij):
    i, j = ij // H, ij % H
    return [k * H + l for k in range(i % 2, H, 2) for l in range(j % 2, H, 2)]


def _layout():
    """Slot/stack assignment for the 625 valid (kl,ij) pairs.

    Returns slots: list of lists of ij; slot t lives at stack t//4,
    partition base 32*(t%4). Each ij occupies rows [off, off+nkl) of its
    slot where off = sum of nkl of earlier ijs in the slot.
    """
    EE = [i * H + j for i in range(0, H, 2) for j in range(0, H, 2)]  # 16x16
    EO = [i * H + j for i in range(0, H, 2) for j in range(1, H, 2)]  # 12x12
    OE = [i * H + j for i in range(1, H, 2) for j in range(0, H, 2)]  # 12x12
    OO = [i * H + j for i in range(1, H, 2) for j in range(1, H, 2)]  # 9x9
    slots = []
    slots += [[EE[2 * t], EE[2 * t + 1]] for t in range(8)]           # 32 rows
    slots += [[EO[2 * t], EO[2 * t + 1]] for t in range(6)]           # 24 rows
    slots += [[OE[2 * t], OE[2 * t + 1]] for t in range(6)]           # 24 rows
    slots += [[OO[3 * t], OO[3 * t + 1], OO[3 * t + 2]] for t in range(3)]
    assert len(slots) == 23
    return slots


SLOTS = _layout()
NSTACK = 6
STACK_K = [128, 128, 128, 128, 128, 97]  # stack 5: 3 slots + bias row at 96


PHASE = os.environ.get("CORR_PHASE", "full")  # gram|perm|fc1|full


# ---------------------------------------------------------------- device IR
def _build():
    dt = BF16
    nc = bass.Bass()

    xh = nc.dram_tensor("xh", [128, NCHUNK * CHW], dt, kind="ExternalInput")
    selh = nc.dram_tensor("selh", [HW, HW * 32], dt, kind="ExternalInput")
    w1h = nc.dram_tensor("w1h", [128, NSTACK * REP], dt, kind="ExternalInput")
    w2h = nc.dram_tensor("w2h", [128, 8 * REP], dt, kind="ExternalInput")
    w3h = nc.dram_tensor("w3h", [128, 8 * 4], dt, kind="ExternalInput")
    b2h = nc.dram_tensor("b2h", [1, REP], dt, kind="ExternalInput")
    b3h = nc.dram_tensor("b3h", [1, 4], dt, kind="ExternalInput")
    onesh = nc.dram_tensor("onesh", [1, 128], dt, kind="ExternalInput")
    identh = nc.dram_tensor("identh", [128, 128], dt, kind="ExternalInput")
    zbh = nc.dram_tensor("zbh", [128, 1], F32, kind="ExternalInput")
    outh = nc.dram_tensor("outh", [128, 4], F32, kind="ExternalOutput")

    from contextlib import ExitStack

    with ExitStack() as ctx:
        sb = lambda name, shape, d: ctx.enter_context(nc.sbuf_tensor(name, shape, d))
        ps = lambda name, shape, d: ctx.enter_context(nc.psum_tensor(name, shape, d))
        sem = lambda name: ctx.enter_context(nc.semaphore(name))

        xs = sb("xs", [128, NCHUNK, CHW], dt)
        gsa = sb("gsa", [HW, BL, HW], dt)
        sel = sb("sel", [HW, HW * 32], dt)
        w1s = sb("w1s", [128, NSTACK, REP], dt)
        w2s = sb("w2s", [128, 8 * REP], dt)
        w3s = sb("w3s", [128, 8 * 4], dt)
        Ssb = sb("Ssb", [128, NSTACK, 128], dt)
        b2s = sb("b2s", [1, REP], dt)
        b3s = sb("b3s", [1, 4], dt)
        ones = sb("ones", [1, 128], dt)
        idents = sb("idents", [128, 128], dt)
        zbias = sb("zbias", [128, 1], F32)
        relu1 = sb("relu1", [128, REP], dt)
        r1T = sb("r1T", [128, 8, 128], dt)
        relu2 = sb("relu2", [128, REP], dt)
        r2T = sb("r2T", [128, 8, 128], dt)
        outs = sb("outs", [128, 4], F32)

        psG0 = ps("psG0", [HW, NG, HW], F32)
        psG1 = ps("psG1", [HW, NG, HW], F32)
        psS0 = ps("psS0", [128, 4, 128], F32)
        psS1 = ps("psS1", [128, 260], F32)  # stacks 4,5 + psO in one bank
        psF0 = ps("psF0", [128, 512], F32)
        psF1 = ps("psF1", [128, 512], F32)
        psT0 = ps("psT0", [128, 128], dt)
        psT1 = ps("psT1", [128, 128], dt)
        psO = psS1[:, 256:260]

        s_x = [sem(f"s_x{i}") for i in range(NCHUNK)]
        s_w2a = sem("s_w2a")
        s_w2b = sem("s_w2b")
        s_sel = sem("s_sel")
        s_wA = sem("s_wA")
        s_wB = sem("s_wB")
        s_wC = sem("s_wC")
        s_wD = sem("s_wD")
        s_g = sem("s_g")
        s_ed = sem("s_ed")
        s_p0 = sem("s_p0")
        s_p1 = sem("s_p1")
        s_se = sem("s_se")
        s_f1 = sem("s_f1")
        s_r1 = sem("s_r1")
        s_t1 = sem("s_t1")
        s_c1 = sem("s_c1")
        s_f2 = sem("s_f2")
        s_r2 = sem("s_r2")
        s_t2 = sem("s_t2")
        s_c2 = sem("s_c2")
        s_f3 = sem("s_f3")
        s_oe = sem("s_oe")
        s_o = sem("s_o")

        block = ctx.enter_context(nc.Block())
        psG = [psG0, psG1]
        psF = [psF0, psF1]
        psT = [psT0, psT1]

        # x slice helpers: chunk layout per partition = [t, h, b, ij]
        def xsl(ch, t, h, b):
            off = ((t * 2 + h) * CB + b) * HW
            return xs[:, ch, off : off + HW]

        # ---------------- SP: even x chunks, w2 half 0, final output
        @block.sync
        def _(sp):
            for ch in range(0, NCHUNK, 2):
                sp.dma_start(
                    xs[:, ch, :], xh[:, ch * CHW : (ch + 1) * CHW]
                ).then_inc(s_x[ch], 16)
            sp.dma_start(w2s[:, 0:4096], w2h[:, 0:4096]).then_inc(s_w2a, 16)
            sp.wait_ge(s_oe, 1)
            sp.dma_start(outh[:, :], outs[:, :]).then_inc(s_o, 16)
            sp.wait_ge(s_o, 16)

        # ---------------- GPSIMD: weights/constants via SWDGE
        @block.gpsimd
        def _(gp):
            gp.dma_start(sel[:, :], selh[:, :]).then_inc(s_sel, 16)
            gp.dma_start(w1s[:, :, :], w1h[:, :]).then_inc(s_wA, 16)
            gp.dma_start(ones[:, :], onesh[:, :]).then_inc(s_wA, 16)
            gp.dma_start(Ssb[96:97, 5, :], onesh[:, :]).then_inc(s_wA, 16)
            gp.dma_start(idents[:, :], identh[:, :]).then_inc(s_wB, 16)
            gp.dma_start(zbias[:, :], zbh[:, :]).then_inc(s_wC, 16)
            gp.dma_start(w3s[:, :], w3h[:, :]).then_inc(s_wD, 16)
            gp.dma_start(b2s[:, :], b2h[:, :]).then_inc(s_wD, 16)
            gp.dma_start(b3s[:, :], b3h[:, :]).then_inc(s_wD, 16)

        # ---------------- ACT: odd x chunks, w2 half 1, psS evicts, relus
        @block.scalar
        def _(act):
            for ch in range(1, NCHUNK, 2):
                act.dma_start(
                    xs[:, ch, :], xh[:, ch * CHW : (ch + 1) * CHW]
                ).then_inc(s_x[ch], 16)
            act.dma_start(w2s[:, 4096:8192], w2h[:, 4096:8192]).then_inc(s_w2b, 16)

            if PHASE == "gram":
                act.wait_ge(s_ed, NGROUP)
                act.activation(
                    outs[0:49, :], gsa[:, 0, 0:4],
                    mybir.ActivationFunctionType.Copy,
                ).then_inc(s_oe, 1)
                return

            # evict stacked gram psum -> S sbuf
            act.wait_ge(s_p0, 1)
            act.activation(
                Ssb[:, 0:4, :], psS0[:, :, :], mybir.ActivationFunctionType.Copy
            ).then_inc(s_se, 1)
            act.wait_ge(s_p1, 1)
            act.activation(
                Ssb[:, 4, :], psS1[:, 0:128], mybir.ActivationFunctionType.Copy
            ).then_inc(s_se, 1)
            act.activation(
                Ssb[0:96, 5, :], psS1[0:96, 128:256], mybir.ActivationFunctionType.Copy
            ).then_inc(s_se, 1)

            if PHASE == "perm":
                act.activation(
                    outs[:, :], Ssb[:, 0, 0:4], mybir.ActivationFunctionType.Copy
                ).then_inc(s_oe, 1)
                return

            act.wait_ge(s_wC, 16)  # zbias
            for hf in range(2):
                act.wait_ge(s_f1, hf + 1)
                act.activation(
                    relu1[:, hf * 512 : (hf + 1) * 512], psF[hf][:, :],
                    mybir.ActivationFunctionType.Relu, bias=zbias[:, :],
                ).then_inc(s_r1, 1)
            if PHASE == "fc1":
                act.activation(
                    outs[:, :], relu1[:, 0:4], mybir.ActivationFunctionType.Copy
                ).then_inc(s_oe, 1)
                return

            for hf in range(2):
                act.wait_ge(s_f2, hf + 1)
                act.activation(
                    relu2[:, hf * 512 : (hf + 1) * 512], psF[hf][:, :],
                    mybir.ActivationFunctionType.Relu, bias=zbias[:, :],
                ).then_inc(s_r2, 1)
            act.wait_ge(s_f3, 1)
            act.activation(
                outs[:, :], psO, mybir.ActivationFunctionType.Copy
            ).then_inc(s_oe, 1)

        # ---------------- PE: all matmuls
        @block.tensor
        def _(pe):
            # gram: G[b]^T[kl, ij] for each local RoI
            for ch in range(NCHUNK):
                pe.wait_ge(s_x[ch], 16)
                for g in range(GPC):
                    gi = ch * GPC + g
                    q = gi % 2
                    if gi >= 2:
                        pe.wait_ge(s_ed, gi - 1)
                    for bb in range(NG):
                        lb = g * NG + bb
                        for h in range(2):
                            mm = pe.matmul(
                                psG[q][:, bb, :],
                                xsl(ch, 1, h, lb),
                                xsl(ch, 0, h, lb),
                                start=(h == 0),
                                stop=(h == 1),
                            )
                    mm.then_inc(s_g, 1)

            if PHASE == "gram":
                return

            # stack: scatter valid (kl,ij) into dense stacks via 0/1 matmuls
            pe.wait_ge(s_ed, NGROUP)
            pe.wait_ge(s_sel, 16)
            for t, ijs in enumerate(SLOTS):
                st, base = t // 4, 32 * (t % 4)
                for u, ij in enumerate(ijs):
                    pst = (
                        psS0[base : base + 32, st, :]
                        if st < 4
                        else psS1[base : base + 32, (st - 4) * 128 : (st - 3) * 128]
                    )
                    mm = pe.matmul(
                        pst,
                        sel[:, ij * 32 : (ij + 1) * 32],
                        gsa[:, :, ij],
                        start=(u == 0),
                        stop=(u == len(ijs) - 1),
                        tile_position=(0, base),
                    )
                if t == 15:
                    mm.then_inc(s_p0, 1)
                if t == 22:
                    mm.then_inc(s_p1, 1)

            if PHASE == "perm":
                return

            # fc1: psF[hf] += S[stack]^T @ W1[stack]
            pe.wait_ge(s_se, 3)
            pe.wait_ge(s_wA, 48)  # w1, ones, S bias row
            for hf in range(2):
                for s in range(NSTACK):
                    ks = STACK_K[s]
                    mm = pe.matmul(
                        psF[hf][:, :],
                        Ssb[0:ks, s, :],
                        w1s[0:ks, s, hf * 512 : hf * 512 + 512],
                        start=(s == 0),
                        stop=(s == NSTACK - 1),
                    )
                mm.then_inc(s_f1, 1)

            if PHASE == "fc1":
                return

            # transpose relu1 -> r1T
            pe.wait_ge(s_wB, 16)  # idents
            for k in range(8):
                pe.wait_ge(s_r1, 1 if k < 4 else 2)
                if k >= 2:
                    pe.wait_ge(s_c1, k - 1)
                pe.transpose(
                    psT[k % 2][:, :], relu1[:, k * 128 : (k + 1) * 128], idents[:, :]
                ).then_inc(s_t1, 1)

            # fc2
            pe.wait_ge(s_w2a, 16)
            pe.wait_ge(s_w2b, 16)
            pe.wait_ge(s_wD, 48)  # w3, b2, b3
            for k in range(8):
                pe.wait_ge(s_c1, k + 1)
                for hf in range(2):
                    pe.matmul(
                        psF[hf][:, :],
                        r1T[:, k, :],
                        w2s[:, k * REP + hf * 512 : k * REP + hf * 512 + 512],
                        start=(k == 0),
                        stop=False,
                    )
            for hf in range(2):
                pe.matmul(
                    psF[hf][:, :],
                    ones[:, :],
                    b2s[:, hf * 512 : hf * 512 + 512],
                    start=False,
                    stop=True,
                ).then_inc(s_f2, 1)

            # transpose relu2 -> r2T
            for k in range(8):
                pe.wait_ge(s_r2, 1 if k < 4 else 2)
                if k >= 2:
                    pe.wait_ge(s_c2, k - 1)
                pe.transpose(
                    psT[k % 2][:, :], relu2[:, k * 128 : (k + 1) * 128], idents[:, :]
                ).then_inc(s_t2, 1)

            # fc3
            for k in range(8):
                pe.wait_ge(s_c2, k + 1)
                pe.matmul(
                    psO,
                    r2T[:, k, :],
                    w3s[:, k * 4 : (k + 1) * 4],
                    start=(k == 0),
                    stop=False,
                )
            pe.matmul(
                psO, ones[:, :], b3s[:, :], start=False, stop=True
            ).then_inc(s_f3, 1)

        # ---------------- DVE: gram evictions + transpose copybacks
        @block.vector
        def _(dve):
            for gi in range(NGROUP):
                q = gi % 2
                dve.wait_ge(s_g, gi + 1)
                dve.tensor_copy(
                    gsa[:, gi * NG : (gi + 1) * NG, :], psG[q][:, :, :]
                ).then_inc(s_ed, 1)
            if PHASE in ("gram", "perm", "fc1"):
                return
            for k in range(8):
                dve.wait_ge(s_t1, k + 1)
                dve.tensor_copy(r1T[:, k, :], psT[k % 2][:, :]).then_inc(s_c1, 1)
            for k in range(8):
                dve.wait_ge(s_t2, k + 1)
                dve.tensor_copy(r2T[:, k, :], psT[k % 2][:, :]).then_inc(s_c2, 1)

    return nc


def _get_nc():
    key = ("nc", PHASE)
    if key not in _CACHE:
        _CACHE[key] = _build()
    return _CACHE[key]


# ---------------------------------------------------------------- host prep
def _prep_weights(W1, b1, np_dt):
    """W1 packed by stack layout + Sel matrices."""
    w1np = np.zeros((128, NSTACK, REP), dtype=np.float32)
    selnp = np.zeros((HW, HW * 32), dtype=np.float32)
    for t, ijs in enumerate(SLOTS):
        st, base = t // 4, 32 * (t % 4)
        off = 0
        for ij in ijs:
            i, j = ij // H, ij % H
            for m, kl in enumerate(_klist(ij)):
                k, l = kl // H, kl % H
                ph = (k - i) // 2 + 7
                pw = (l - j) // 2 + 7
                f = (ph * P + pw) * HW + ij
                w1np[base + off + m, st, :] = W1[:, f]
                selnp[kl, ij * 32 + off + m] = 1.0
            off += len(_klist(ij))
    w1np[96, 5, :] = b1
    return w1np.reshape(128, NSTACK * REP).astype(np_dt), selnp.astype(np_dt)


# ---------------------------------------------------------------- entry
def kernel(patch1, patch2, W1, b1, W2, b2, W3, b3):
    global LAST_EXEC_NS
    import ml_dtypes

    np_dt = ml_dtypes.bfloat16

    patch1 = np.asarray(patch1, dtype=np.float32).reshape(B, 2, 128, HW)
    patch2 = np.asarray(patch2, dtype=np.float32).reshape(B, 2, 128, HW)
    W1 = np.asarray(W1, dtype=np.float32)
    W2 = np.asarray(W2, dtype=np.float32)
    W3 = np.asarray(W3, dtype=np.float32)
    b1 = np.asarray(b1, dtype=np.float32)
    b2 = np.asarray(b2, dtype=np.float32)
    b3 = np.asarray(b3, dtype=np.float32)

    w1e, sele = _prep_weights(W1, b1, np_dt)
    w2e = np.ascontiguousarray(
        W2.T.reshape(8, 128, REP).transpose(1, 0, 2).reshape(128, 8 * REP)
    ).astype(np_dt)
    w3e = np.ascontiguousarray(
        W3.T.reshape(8, 128, 4).transpose(1, 0, 2).reshape(128, 32)
    ).astype(np_dt)

    shared = {
        "selh": sele,
        "w1h": w1e,
        "w2h": w2e,
        "w3h": w3e,
        "b2h": b2.reshape(1, REP).astype(np_dt),
        "b3h": b3.reshape(1, 4).astype(np_dt),
        "onesh": np.ones((1, 128), dtype=np_dt),
        "identh": np.eye(128, dtype=np.float32).astype(np_dt),
        "zbh": np.zeros((128, 1), dtype=np.float32),
    }

    in_maps = []
    for i in range(N_CORES):
        sl = slice(i * BL, (i + 1) * BL)
        # xh[c, ch, t, h, b, ij]
        xt = np.stack([patch1[sl], patch2[sl]], axis=0)  # [t, 128b, h, c, ij]
        xt = xt.reshape(2, NCHUNK, CB, 2, 128, HW).transpose(4, 1, 0, 3, 2, 5)
        xh = np.ascontiguousarray(xt).reshape(128, NCHUNK * CHW).astype(np_dt)
        in_maps.append({"xh": xh, **shared})

    nc = _get_nc()
    trace = os.environ.get("CORR_TRACE", "0") == "1"
    res = run_bass_kernel_spmd(nc, in_maps, list(range(N_CORES)), trace=trace)
    LAST_EXEC_NS = res.exec_time_ns

    out = np.concatenate(
        [res.results[i]["outh"] for i in range(N_CORES)], axis=0
    ).astype(np.float32)
    return out
